# revision 102
# baseline (speedup 1.0000x reference)
"""DSS (Diagonal State Space) layer as a Bass/Tile kernel for 8 Trainium2 NeuronCores.

Algorithm (per core, channels H sharded 8 x 128):
  1. Build the DSS-exp kernel k[l,h] = Re(sum_n W[h,n] z[h,n]^l), z = exp(dt_h * Lambda_n),
     on-device via a two-level power factorization l = 32a + b:
       GW[h,n,b] = W * z^b (b<32),  Z32[h,n,a] = z^(32a) (a<16), both by complex doubling,
     then a per-channel PE matmul contracts the 64 modes. The whole prologue runs in a
     split-H layout (partitions = (h//64, n)) so every elementwise op uses all 128
     partitions; the two power chains run concurrently on DVE (GW) and Pool (Z32).
  2. K_f = rfft_1024(k) via PE matmuls against host-precomputed DFT tiles.
  3. Overlap-save FFT convolution with HALF-WINDOW SHARING: each 512-sample half-window's
     1024-point DFT contribution P_m is computed once (4 chunk matmuls x 2 parts x 4 freq
     tiles) and reused by the two blocks that contain it: U_b = P_{b-1} + (-1)^f P_b.
     Frequencies are packed even/odd (tiles 0,1 = even f, tiles 2,3 = odd f) so the
     (-1)^f factor is a constant +1/-1 per tile and the combine is one bf16 add/subtract.
     Pointwise complex multiply runs in bf16 (2x DVE rate); inverse rfft as PE matmuls
     against bf16 stationaries producing the valid 512 samples.
  4. The skip connection y += u * D is folded into the frequency-domain filter
     (K'_f = K_f + D), so it costs nothing in the main loop.

The main data path (u chunks, DFT stationaries, spectra, K tiles) is bf16 with fp32 PSUM
accumulation, which keeps the relative error ~7e-3 under the 2e-2 gate while doubling
DVE pointwise rate and halving DMA/SBUF.

Schedule (timeline-profiled): six forward halves are emitted up front so PE runs the
forward stream under the prologue's transcendental chains (DVE/Pool, fused-STT Horner);
the mode-sum, kdft and K assembly chase the forward with no PE gap; the 8-block main
loop is DVE-bound (~7.6us/block: 4 wide combines + 18 bf16 pointwise ops vs Pool's 6)
with halves 6/7 recomputed mid-loop where the P-ring has free slots. Forward cos/sin
PSUM pairs share a 2-bank tile with one wide ACT evacuation; chunk/constant DMAs are
interleaved in first-use order across both hardware queues (kcb also uses the Pool
queue). 141.6us on the timeline model vs 252.7us baseline.
"""

import sys

for _p in ("/opt/trn_rl_repo", "/opt/trn_rl_repo/concourse"):
    if _p not in sys.path:
        sys.path.insert(0, _p)

import numpy as np
import ml_dtypes
from contextlib import ExitStack

import concourse.bacc as bacc
import concourse.tile as tile
import concourse.mybir as mybir

dt = mybir.dt
f32 = np.float32
bf16 = ml_dtypes.bfloat16

B, L, H, N = 4, 4096, 1024, 64
LK = 512
F = 1024          # FFT length (overlap-save)
HOP = 512         # block hop
NCORES = 8
HS = H // NCORES  # 128 channels per core
NBLK = L // HOP   # 8
NFT = 4           # packed frequency tiles (even: 0,1; odd: 2,3; Nyquist folded)
NJ = 4            # contraction chunks per half-window
NLT = HOP // 128  # 4 output l-tiles per block
NCH = L // 128    # 32 u chunks per core
NHALF = L // HOP  # 8 half-windows
HG = 64           # channels per split-H partition group (prologue layout)


# ---------------------------------------------------------------- host constants
def _freq_perm():
    perm = np.zeros((NFT, 128), dtype=np.int64)
    r = np.arange(128)
    perm[0] = 2 * r
    perm[1] = 2 * (128 + r)
    perm[2] = 2 * r + 1
    perm[3] = 2 * (128 + r) + 1
    return perm


def build_constants():
    perm = _freq_perm()
    l = np.arange(HOP, dtype=np.float64)
    CF = np.zeros((NJ, 2, NFT, 128, 128))
    for j in range(NJ):
        lj = l[128 * j:128 * j + 128][:, None]
        for ft in range(NFT):
            f = perm[ft][None, :].astype(np.float64)
            ang = 2 * np.pi * lj * f / F
            CF[j, 0, ft] = np.cos(ang)
            CF[j, 1, ft] = -np.sin(ang)
        # Nyquist row packed into the f=0 slot of the sin tile (even group, sign +1)
        CF[j, 1, 0][:, 0] = (-1.0) ** l[128 * j:128 * j + 128]
    lc = HOP + np.arange(HOP, dtype=np.float64)[None, :]   # valid circular outputs
    AI = np.zeros((2, NFT, NLT, 128, 128))
    for ft in range(NFT):
        fr = perm[ft][:, None].astype(np.float64)
        cf_ = np.where(fr == 0, 1.0, 2.0)
        Ar = cf_ * np.cos(2 * np.pi * fr * lc / F) / F
        Ai = -(2.0 / F) * np.sin(2 * np.pi * fr * lc / F)
        if ft == 0:
            Ai[0, :] = ((-1.0) ** lc[0]) / F                # Nyquist inverse row
        for lt in range(NLT):
            AI[0, ft, lt] = Ar[:, 128 * lt:128 * lt + 128]
            AI[1, ft, lt] = Ai[:, 128 * lt:128 * lt + 128]
    return CF.astype(bf16), AI.astype(bf16)


# Horner coefficient lists (highest degree first)
def _fact(k):
    r = 1.0
    for i in range(2, k + 1):
        r *= i
    return r


EXP10 = [1.0 / _fact(k) for k in range(7, -1, -1)]          # e^x, |x| <~ 0.9
EXP9 = [1.0 / _fact(k) for k in range(6, -1, -1)]            # e^x, |x| <~ 0.4
SIN9 = [1.0 / _fact(9), -1.0 / _fact(7), 1.0 / _fact(5), -1.0 / _fact(3), 1.0]   # odd, in u = x^2
COSC = [-1.0 / _fact(10), 1.0 / _fact(8), -1.0 / _fact(6), 1.0 / _fact(4),
        -1.0 / _fact(2), 1.0]     # cos(x) - 1 in u = x^2, b1 first


class _Prog:
    def __init__(self):
        self.nc = None
        self.built = False


_prog = _Prog()


def _emit_kernel(nc, tc, ctx, aps):
    V = nc.vector
    A = nc.scalar
    T = nc.tensor
    GP = nc.gpsimd
    u_ap = aps["u"]; y_ap = aps["y"]
    cf_ap = aps["CF"]; ai_ap = aps["AI"]
    TT = V.tensor_tensor
    GT = GP.tensor_tensor
    op = mybir.AluOpType

    # ---------------- pools
    p_cf = ctx.enter_context(tc.tile_pool(name="cf", bufs=1))
    p_ai = ctx.enter_context(tc.tile_pool(name="ai", bufs=1))
    p_uch = ctx.enter_context(tc.tile_pool(name="uch", bufs=10))
    p_pm = ctx.enter_context(tc.tile_pool(name="pm", bufs=24))
    p_u = ctx.enter_context(tc.tile_pool(name="u", bufs=10))
    p_yf = ctx.enter_context(tc.tile_pool(name="yf", bufs=16))
    p_krep = ctx.enter_context(tc.tile_pool(name="krep", bufs=1))
    p_tmp = ctx.enter_context(tc.tile_pool(name="tmp", bufs=9))
    p_yout = ctx.enter_context(tc.tile_pool(name="yout", bufs=3))
    p_kc = ctx.enter_context(tc.tile_pool(name="kc", bufs=4))
    p_gw = ctx.enter_context(tc.tile_pool(name="gw", bufs=1))
    p_z32 = ctx.enter_context(tc.tile_pool(name="z32", bufs=1))
    p_zp = ctx.enter_context(tc.tile_pool(name="zp", bufs=9))
    p_small = ctx.enter_context(tc.tile_pool(name="small", bufs=1))
    p_gwtmp = ctx.enter_context(tc.tile_pool(name="gwtmp", bufs=1))
    p_drep = ctx.enter_context(tc.tile_pool(name="drep", bufs=1))
    p_ks = ctx.enter_context(tc.tile_pool(name="ks", bufs=2))
    p_psf = ctx.enter_context(tc.tile_pool(name="psf", bufs=2, space="PSUM"))
    p_psi = ctx.enter_context(tc.tile_pool(name="psi", bufs=2, space="PSUM"))
    p_psk = ctx.enter_context(tc.tile_pool(name="psk", bufs=2, space="PSUM"))

    # ---------------- small parameter tiles first (they gate the whole k prologue)
    logdt = p_small.tile([1, HS], dt.float32, tag="logdt")
    A.dma_start(logdt[:], aps["logdt"][:])
    Lre = p_small.tile([1, N], dt.float32, tag="lre")
    A.dma_start(Lre[:], aps["Lre"][:])
    Lim_r = p_small.tile([1, N], dt.float32, tag="lim")
    A.dma_start(Lim_r[:], aps["Lim"][:])
    # GW seed planes load early on the SP queue (declared here, queued just
    # after the first cf/chunk loads): the ACT engine queue fills with
    # half-spectrum evacuations, which would delay the power chains to ~60us
    GWre_r = p_gw.tile([128, HG, 32], dt.float32r, tag="gwre")
    GWim_r = p_gw.tile([128, HG, 32], dt.float32r, tag="gwim")   # stores -Im
    GWre = GWre_r[:]
    GWim = GWim_r[:]
    wimt = p_small.tile([128, HG], dt.float32, tag="wimt")

    # ---------------- constant stationaries + u chunks, interleaved in first-use
    # order so the first forward matmul is not head-of-line blocked
    cf_big = {}
    _cfq = [0]

    def load_cf(j, t_):
        tl = p_cf.tile([128, 4, 128], dt.bfloat16, tag=f"cf{j}_{t_}", name=f"cfb{j}_{t_}")
        eng = (nc.sync, nc.scalar)[_cfq[0] % 2]
        _cfq[0] += 1
        eng.dma_start(tl[:], cf_ap[j, t_].transpose([1, 0, 2]))
        cf_big[(j, t_)] = tl

    def cf_tile(j, t_, ft):
        return cf_big[(j, t_)][:, ft, :]

    chunks = {}

    def get_chunk(c):
        if c not in chunks:
            t_u = p_uch.tile([128, 4, 128], dt.bfloat16, tag="uch", name=f"uch{c}")
            # chunks 2/3 ride the otherwise-idle Pool queue so the first
            # half-window is not serialized behind the constant loads
            if c in (1, 2):
                eng = GP
            else:
                eng = nc.sync if c % 2 == 0 else nc.scalar
            eng.dma_start(t_u[:], u_ap[:, 128 * c:128 * c + 128, :].transpose([1, 0, 2]))
            chunks[c] = t_u
        return chunks[c]

    for j in range(NJ):
        load_cf(j, 0)
        load_cf(j, 1)
        get_chunk(2 * j)
        get_chunk(2 * j + 1)
        if j == 0:
            nc.sync.dma_start(GWre[:, :, 0], aps["Wre"][:].bitcast(dt.float32r))
            nc.sync.dma_start(wimt[:], aps["Wim"][:])

    # ---------------- forward: one half-window spectrum P_m per 512 samples
    halves = {}

    def emit_half(m, fts=range(NFT)):
        # cos/sin accumulation groups share one 2-bank psum tile per ft, so the
        # evacuation is a single wide ACT copy (the fwd would otherwise be
        # ACT-paced: 2x612ns of copies per 852ns of matmuls)
        out = halves.get(m, [])
        for ft in fts:
            pp = p_psf.tile([128, 1024], dt.float32, tag="psf", name=f"pp{m}_{ft}")
            for j in range(NJ):
                ch = get_chunk(4 * m + j)
                T.matmul(pp[:, 0:512], cf_tile(j, 0, ft), ch[:].rearrange("p b h -> p (b h)"),
                         start=(j == 0), stop=(j == NJ - 1))
                T.matmul(pp[:, 512:1024], cf_tile(j, 1, ft), ch[:].rearrange("p b h -> p (b h)"),
                         start=(j == 0), stop=(j == NJ - 1))
            pm = p_pm.tile([128, 1024], dt.bfloat16, tag="pm", name=f"pm{m}_{ft}")
            A.copy(pm[:], pp[:])
            out.append(pm)
        halves[m] = out

    # half of half-0 keeps PE busy while the k-prologue's parameter chains run;
    # the outer-product matmuls slot in right after (~13us, as soon as their dtv
    # input is ready) so the trig chains are not stuck behind the forward stream
    emit_half(0, fts=range(0, 2))

    def horner_exp(dst, x, coefs, eng=None, tag="horner"):
        # dst = e^x = 1 + x*Q(x). On DVE each Horner step p <- (p + b)*x is one
        # fused scalar_tensor_tensor op; the Pool engine has no STT on hardware,
        # so it uses the classic two-op form there.
        tsm = eng.tensor_scalar_mul if eng is not None else V.tensor_scalar_mul
        tsa = eng.tensor_scalar_add if eng is not None else V.tensor_scalar_add
        p = dst
        tsm(p, x, float(coefs[0]))
        for c in coefs[1:-1]:
            if eng is None:
                V.scalar_tensor_tensor(p, p, float(c), x, op.add, op.mult)
            else:
                tsa(p, p, float(c))
                eng.tensor_tensor(p, p, x, op.mult)
        tsa(p, p, 1.0)

    # dt = exp(logdt) = (exp(logdt/8))^8   (DVE chain)
    x8 = p_small.tile([1, HS], dt.float32, tag="x8")
    V.tensor_scalar_mul(x8[:], logdt[:], 0.125)
    e8 = p_small.tile([1, HS], dt.float32, tag="e8")
    horner_exp(e8[:], x8[:], EXP10, tag="h_dt")
    dtv = p_small.tile([1, HS], dt.float32, tag="dtv")
    t_a = p_small.tile([1, HS], dt.float32, tag="sq1")
    TT(t_a[:], e8[:], e8[:], op.mult)
    t_b = p_small.tile([1, HS], dt.float32, tag="sq2")
    TT(t_b[:], t_a[:], t_a[:], op.mult)
    TT(dtv[:], t_b[:], t_b[:], op.mult)

    # -exp(Lre) = -(exp(Lre/8))^8   (Pool chain, concurrent with the dt chain)
    xl = p_small.tile([1, N], dt.float32, tag="xl")
    GP.tensor_scalar_mul(xl[:], Lre[:], 0.125)
    el8 = p_small.tile([1, N], dt.float32, tag="el8")
    horner_exp(el8[:], xl[:], EXP10, eng=GP, tag="h_lre")
    t_c = p_small.tile([1, N], dt.float32, tag="sq3")
    GT(t_c[:], el8[:], el8[:], op.mult)
    t_d = p_small.tile([1, N], dt.float32, tag="sq4")
    GT(t_d[:], t_c[:], t_c[:], op.mult)
    negel = p_small.tile([1, N], dt.float32, tag="negel")
    t_e = p_small.tile([1, N], dt.float32, tag="sq5")
    GT(t_e[:], t_d[:], t_d[:], op.mult)
    GP.tensor_scalar_mul(negel[:], t_e[:], -1.0)

    # outer products in split-H layout: partitions = (hg, n) with hg = h // 64,
    # so every later [*, HG] elementwise op uses all 128 partitions (half the
    # free-size cost of the old [N, HS] layout)
    ps_a = p_psk.tile([128, 512], dt.float32, tag="psk")
    ps_b = p_psk.tile([128, 512], dt.float32, tag="psk")
    for hg in range(2):
        T.matmul(ps_a[64 * hg:64 * hg + 64, 0:HG], negel[:],
                 dtv[:, HG * hg:HG * hg + HG], start=True, stop=True)
        T.matmul(ps_b[64 * hg:64 * hg + 64, 0:HG], Lim_r[:],
                 dtv[:, HG * hg:HG * hg + HG], start=True, stop=True)

    # remaining forward halves + inverse stationaries queue behind the outer products
    emit_half(0, fts=range(2, NFT))
    for m in range(1, 6):
        emit_half(m)
    ai_big = {}
    for t_ in range(2):
        for ft in range(NFT):
            tl = p_ai.tile([128, 4, 128], dt.bfloat16, tag=f"ai{t_}_{ft}", name=f"aib{t_}_{ft}")
            eng = nc.sync if (ft + t_) % 2 == 0 else nc.scalar
            eng.dma_start(tl[:], ai_ap[t_, ft].transpose([1, 0, 2]))
            ai_big[(t_, ft)] = tl

    def ai_tile(t_, ft, lt):
        return ai_big[(t_, ft)][:, lt, :]

    # half-angle pieces on [128, HG]; the e^a chain runs on Pool, trig on DVE
    ah = p_small.tile([128, HG], dt.float32, tag="ah")
    V.tensor_scalar_mul(ah[:], ps_a[0:128, 0:HG], 0.5)   # DVE: Pool cannot read PSUM
    bh = p_small.tile([128, HG], dt.float32, tag="bh")
    V.tensor_scalar_mul(bh[:], ps_b[0:128, 0:HG], 0.5)
    ea = p_small.tile([128, HG], dt.float32, tag="ea")
    horner_exp(ea[:], ah[:], EXP9, eng=GP, tag="h_ea")
    # sin(bh), cos(bh) via u = bh^2
    ub = p_small.tile([128, HG], dt.float32, tag="ub")
    TT(ub[:], bh[:], bh[:], op.mult)
    # sin(x)/x = 1 + N(u), cos(x) = 1 + M(u): fused-STT Horner on the
    # zero-constant parts (coeff lists are lowest-power-last, b1 first)
    sp = p_small.tile([128, HG], dt.float32, tag="sp")
    V.tensor_scalar_mul(sp[:], ub[:], float(SIN9[0]))
    for c in SIN9[1:-1]:
        V.scalar_tensor_tensor(sp[:], sp[:], float(c), ub[:], op.add, op.mult)
    V.tensor_scalar_add(sp[:], sp[:], 1.0)
    sb = p_small.tile([128, HG], dt.float32, tag="sb")
    TT(sb[:], sp[:], bh[:], op.mult)          # sin(b/2)
    cb = p_small.tile([128, HG], dt.float32, tag="cb")
    V.tensor_scalar_mul(cb[:], ub[:], float(COSC[0]))
    for c in COSC[1:-1]:
        V.scalar_tensor_tensor(cb[:], cb[:], float(c), ub[:], op.add, op.mult)
    V.tensor_scalar_add(cb[:], cb[:], 1.0)

    wre = p_small.tile([128, HG], dt.float32, tag="wre")
    TT(wre[:], ea[:], cb[:], op.mult)
    wim = p_small.tile([128, HG], dt.float32, tag="wim")
    TT(wim[:], ea[:], sb[:], op.mult)

    # complex squaring on separate re/im planes (all base-partition 0, lane-aligned)
    def csq_parts(dre, dim_, sre, sim, eng=None):
        # re' = (re+im)(re-im), im' = (2 re) im -- 4-5 ops (STT fused on DVE;
        # the Pool engine has no STT on hardware)
        tt = eng.tensor_tensor if eng is not None else TT
        sfx = "p" if eng is not None else ""
        t1 = p_small.tile([128, HG], dt.float32, tag=f"csq1{sfx}", bufs=2)
        tt(t1[:], sre, sim, op.add)
        t2 = p_small.tile([128, HG], dt.float32, tag=f"csq2{sfx}", bufs=2)
        tt(t2[:], sre, sim, op.subtract)
        if eng is None:
            V.scalar_tensor_tensor(dim_, sre, 2.0, sim, op.mult, op.mult)
        else:
            t3 = p_small.tile([128, HG], dt.float32, tag=f"csq3{sfx}", bufs=2)
            tt(t3[:], sre, sim, op.mult)
            eng.tensor_scalar_mul(dim_, t3[:], 2.0)
        tt(dre, t1[:], t2[:], op.mult)

    def new_zpair(nm):
        zr = p_zp.tile([128, HG], dt.float32, tag="zp", name=f"{nm}r")
        zi = p_zp.tile([128, HG], dt.float32, tag="zp", name=f"{nm}i")
        return zr, zi

    # GW plane 0 imaginary part (negated) from the pre-loaded Wim
    V.tensor_scalar_mul(GWim[:, :, 0], wimt[:], -1.0)

    def cdouble_seg(pre, pim, zr, zi, s0, d0, w, conj_stored, prim=None, sec=None):
        # planes [.., d0:d0+w] = planes[.., s0:s0+w] * (zr + i zi);
        # when conj_stored, the im plane holds the negated imaginary part.
        ptt = prim.tensor_tensor if prim is not None else TT
        stt = sec.tensor_tensor if sec is not None else GT
        tg = "zt2" if prim is not None else "gt2"
        zre = zr[:].unsqueeze(2).broadcast_to([128, HG, w])
        zim = zi[:].unsqueeze(2).broadcast_to([128, HG, w])
        t2 = p_gwtmp.tile([128, HG, 8], dt.float32, tag=tg, bufs=2)
        t4 = p_gwtmp.tile([128, HG, 8], dt.float32, tag=tg, bufs=2)
        ptt(pre[:, :, d0:d0 + w], pre[:, :, s0:s0 + w], zre, op.mult)
        stt(t2[:, :, 0:w], pim[:, :, s0:s0 + w], zim, op.mult)
        ptt(pim[:, :, d0:d0 + w], pim[:, :, s0:s0 + w], zre, op.mult)
        stt(t4[:, :, 0:w], pre[:, :, s0:s0 + w], zim, op.mult)
        ptt(pre[:, :, d0:d0 + w], pre[:, :, d0:d0 + w], t2[:, :, 0:w],
            op.add if conj_stored else op.subtract)
        ptt(pim[:, :, d0:d0 + w], pim[:, :, d0:d0 + w], t4[:, :, 0:w],
            op.subtract if conj_stored else op.add)

    def cdouble(pre, pim, zr, zi, w, conj_stored, prim=None, sec=None):
        cdouble_seg(pre, pim, zr, zi, 0, w, w, conj_stored, prim=prim, sec=sec)

    # ---------------- Z32 planes [(hg n), HG, 16] natural complex z^(32a)
    Zre_r = p_z32.tile([128, HG, 16], dt.float32r, tag="z32re")
    Zim_r = p_z32.tile([128, HG, 16], dt.float32r, tag="z32im")
    Zre = Zre_r[:]
    Zim = Zim_r[:]
    # a=0 plane is the complex constant 1+0i (memset can't emit float32r)
    V.tensor_scalar(Zre[:, :, 0], wre[:], 0.0, 1.0, op.mult, op.add)
    V.tensor_scalar(Zim[:, :, 0], wre[:], 0.0, 0.0, op.mult, op.add)

    # interleaved power chain + doubling: GW level j follows zp[j] immediately,
    # Z32 level j follows za[j]. GW doubling runs DVE-primary / Pool-secondary;
    # the Z32 chain runs Pool-primary / DVE-secondary so the chains overlap.
    zp = []
    z0 = new_zpair("z0")
    csq_parts(z0[0][:], z0[1][:], wre[:], wim[:])
    zp.append(z0)
    cdouble(GWre, GWim, zp[0][0], zp[0][1], 1, conj_stored=True)
    for j in range(1, 5):                     # z^2, z^4, z^8, z^16
        zj = new_zpair(f"z{1 << j}")
        csq_parts(zj[0][:], zj[1][:], zp[-1][0][:], zp[-1][1][:])
        zp.append(zj)
        if j < 4:
            cdouble(GWre, GWim, zp[j][0], zp[j][1], 1 << j, conj_stored=True)
    za = []
    z32t = new_zpair("z32")
    csq_parts(z32t[0][:], z32t[1][:], zp[4][0][:], zp[4][1][:], eng=GP)
    za.append(z32t)                           # z^32
    cdouble_seg(GWre, GWim, zp[4][0], zp[4][1], 0, 16, 8, conj_stored=True)
    cdouble_seg(GWre, GWim, zp[4][0], zp[4][1], 8, 24, 8, conj_stored=True)
    cdouble(Zre, Zim, za[0][0], za[0][1], 1, conj_stored=False, prim=GP, sec=V)
    for j in range(1, 4):                     # z^64, z^128, z^256
        zj = new_zpair(f"za{j}")
        csq_parts(zj[0][:], zj[1][:], za[-1][0][:], za[-1][1][:], eng=GP)
        za.append(zj)
        cdouble(Zre, Zim, za[j][0], za[j][1], 1 << j, conj_stored=False, prim=GP, sec=V)

    # ---------------- mode-sum: k[32a+b, h], two contraction-64 matmuls per channel
    # psum += GWre_h^T @ Zre_h ; psum += GWim_h^T @ Zim_h  (im plane is negated)
    ks = None
    for g in range(4):
        kp_g = p_psk.tile([32, 32, 16], dt.float32, tag="psk", name=f"kp{g}")
        for hl in range(32):
            h = 32 * g + hl
            hg, hp = h // HG, h % HG
            T.matmul(kp_g[0:32, hl, :], GWre_r[64 * hg:64 * hg + 64, hp, :],
                     Zre_r[64 * hg:64 * hg + 64, hp, :], start=True, stop=False)
            T.matmul(kp_g[0:32, hl, :], GWim_r[64 * hg:64 * hg + 64, hp, :],
                     Zim_r[64 * hg:64 * hg + 64, hp, :], start=False, stop=True)
        # evacuate (a-major, all channels in one tile) on DVE; the kdft below
        # contracts 32 taps at a time directly from this layout, so no DMA
        # shuffle into l-major tiles is needed at all
        if g == 0:
            ks = p_ks.tile([32, 16, 128], dt.bfloat16, tag="ks", name="ksall")
        V.tensor_copy(ks[0:32, :, 32 * g:32 * g + 32], kp_g[:].transpose([0, 2, 1]))
    # shuffle into one l-major tile for a contraction-128 kdft: 4 partition-subrange
    # DMAs, each moving one al row-block for all chunks and channel groups
    kcb = p_kc.tile([128, 4, 128], dt.bfloat16, tag="kc", name="kcb")
    for al in range(4):
        eng = (nc.sync, nc.scalar, GP, nc.sync)[al]
        eng.dma_start(kcb[:][32 * al:32 * al + 32, :, :],
                      ks[0:32, al:16:4, :])

    # ---------------- K_f via packed DFT (reuse forward stationaries j=0..3).
    # The skip connection u*D folds into the filter as K'_f = K_f + D: a rank-1
    # ones^T x D matmul accumulated into each cos psum group, so the K tiles
    # evacuate with plain ACT copies.
    dtile = p_small.tile([1, HS], dt.float32, tag="dtile")
    A.dma_start(dtile[:], aps["D"][:])
    dt16 = p_small.tile([1, HS], dt.bfloat16, tag="dt16")
    V.tensor_copy(dt16[:], dtile[:])
    ones = p_small.tile([1, 128], dt.bfloat16, tag="ones")
    V.memset(ones[:], 1.0)
    kdft_ps = {}
    pks = {}
    for t_ in range(2):
        pks[t_] = p_psk.tile([128, 4, 128], dt.float32, tag="psk", name=f"kdft{t_}")
    for ft in range(NFT):
        for t_ in range(2):
            for c in range(4):
                T.matmul(pks[t_][:, ft, :], cf_tile(c, t_, ft), kcb[:, c, :],
                         start=(c == 0), stop=(c == 3 and t_ == 1))
            if t_ == 0:
                # fold the skip connection: K'_f = K_f + D on every cos row
                T.matmul(pks[t_][:, ft, :], ones[:], dt16[:], start=False, stop=True)
            kdft_ps[(t_, ft)] = pks[t_][:, ft, :]

    zrow = p_small.tile([1, 128], dt.float32, tag="zrow")
    V.memset(zrow[:], 0.0)
    # ft0's K tiles assemble first -- the first pointwise needs krA[0], krBC[0]
    # AND krD0, so krD0 cannot trail the other copies
    krA, krBC = [], []
    krD0 = p_krep.tile([128, 128], dt.bfloat16, tag="krD0")
    for ft in range(NFT):
        ta = p_krep.tile([128, 128], dt.bfloat16, tag=f"krA{ft}")
        tb = p_krep.tile([128, 128], dt.bfloat16, tag=f"krB{ft}")
        A.copy(ta[:], kdft_ps[(0, ft)])
        A.copy(tb[:], kdft_ps[(1, ft)])
        krA.append(ta)
        krBC.append(tb)
        if ft == 0:
            A.copy(krD0[:], kdft_ps[(0, 0)])
            # row 0 of the D-tensor holds K512r (packed sin psum row 0) plus D
            TT(krD0[0:1, :], kdft_ps[(1, 0)][0:1, :], dtile[:], op.add)
            V.tensor_scalar(krBC[0][0:1, :], zrow[:], 0.0, 0.0, op.mult, op.add)

    # ---------------- main loop: overlap-save blocks

    def kb(t):
        return t[:].unsqueeze(1).broadcast_to([128, 4, 128])

    for blk in range(NBLK):
        # late forward halves: their pm-ring slots reuse halves 0/1, which die
        # at the combines of blocks 1/2, so emission here is deadlock-free
        if blk == 2:
            emit_half(6)
        elif blk == 3:
            emit_half(7)

        # phase A (per ft): combines + the four products; phase B: the two sums.
        # DVE's stream front-loads work it can do without Pool results, so it
        # never stalls on Pool's slower multiplies.
        yr_t, yi_t = [], []
        parts = []
        for ft in range(NFT):
            # combine half spectra (cos and sin at once):
            # U_b = P_{b-1} + P_b (even tiles) / P_{b-1} - P_b (odd)
            pcur = halves[blk][ft]
            if blk == 0:
                if ft < 2:
                    up = pcur
                else:
                    up = p_u.tile([128, 1024], dt.bfloat16, tag="u", name=f"u{blk}_{ft}")
                    V.tensor_scalar_mul(up[:], pcur[:], -1.0)
            else:
                pprev = halves[blk - 1][ft]
                cop = op.add if ft < 2 else op.subtract
                up = p_u.tile([128, 1024], dt.bfloat16, tag="u", name=f"u{blk}_{ft}")
                TT(up[:], pprev[:], pcur[:], cop)
            uc3 = up[:, 0:512].rearrange("p (b h) -> p b h", b=4)
            us3 = up[:, 512:1024].rearrange("p (b h) -> p b h", b=4)

            # pointwise products: Yr = Uc*A - Us*BC ; Yi = Uc*BC + Us*D
            dten = krD0 if ft == 0 else krA[ft]
            t1 = p_tmp.tile([128, 512], dt.bfloat16, tag="t1")
            t2 = p_tmp.tile([128, 512], dt.bfloat16, tag="t2")
            TT(t1[:].rearrange("p (b h) -> p b h", b=4), uc3, kb(krA[ft]), op.mult)
            GT(t2[:].rearrange("p (b h) -> p b h", b=4), us3, kb(krBC[ft]), op.mult)
            t3 = p_tmp.tile([128, 512], dt.bfloat16, tag="t1")
            t4 = p_tmp.tile([128, 512], dt.bfloat16, tag="t2")
            TT(t4[:].rearrange("p (b h) -> p b h", b=4), us3, kb(dten), op.mult)
            # DVE:Pool op ratio tuned to their 327:1111 ns/op costs
            t3eng = TT if ft >= 2 else GT
            t3eng(t3[:].rearrange("p (b h) -> p b h", b=4), uc3, kb(krBC[ft]), op.mult)
            parts.append((t1, t2, t3, t4))
        for ft in range(NFT):
            t1, t2, t3, t4 = parts[ft]
            yr = p_yf.tile([128, 512], dt.bfloat16, tag="yf")
            TT(yr[:], t1[:], t2[:], op.subtract)
            yi = p_yf.tile([128, 512], dt.bfloat16, tag="yf")
            TT(yi[:], t3[:], t4[:], op.add)
            yr_t.append(yr)
            yi_t.append(yi)
        for lt in range(NLT):
            py = p_psi.tile([128, 512], dt.float32, tag="psi")
            for ft in range(NFT):
                T.matmul(py[:], ai_tile(0, ft, lt), yr_t[ft][:],
                         start=(ft == 0), stop=False)
                T.matmul(py[:], ai_tile(1, ft, lt), yi_t[ft][:],
                         start=False, stop=(ft == NFT - 1))
            c_out = 4 * blk + lt
            yo = p_yout.tile([128, 512], dt.float32, tag="yout")
            A.copy(yo[:], py[:])
            eng = nc.sync if lt % 2 == 0 else nc.scalar
            eng.dma_start(y_ap[:, 128 * c_out:128 * c_out + 128, :].transpose([1, 0, 2]),
                          yo[:].rearrange("p (b h) -> p b h", b=4))


def _build_program():
    if _prog.built:
        return
    nc = bacc.Bacc("TRN2", target_bir_lowering=False, debug=False,
                   num_devices=NCORES)
    aps = {}
    aps["u"] = nc.dram_tensor("u", [B, L, HS], dt.bfloat16, kind="ExternalInput").ap()
    aps["D"] = nc.dram_tensor("D", [1, HS], dt.float32, kind="ExternalInput").ap()
    aps["logdt"] = nc.dram_tensor("logdt", [1, HS], dt.float32, kind="ExternalInput").ap()
    aps["Wre"] = nc.dram_tensor("Wre", [128, HG], dt.float32, kind="ExternalInput").ap()
    aps["Wim"] = nc.dram_tensor("Wim", [128, HG], dt.float32, kind="ExternalInput").ap()
    aps["Lre"] = nc.dram_tensor("Lre", [1, N], dt.float32, kind="ExternalInput").ap()
    aps["Lim"] = nc.dram_tensor("Lim", [1, N], dt.float32, kind="ExternalInput").ap()
    aps["CF"] = nc.dram_tensor("CF", [NJ, 2, NFT, 128, 128], dt.bfloat16,
                               kind="ExternalInput").ap()
    aps["AI"] = nc.dram_tensor("AI", [2, NFT, NLT, 128, 128], dt.bfloat16,
                               kind="ExternalInput").ap()
    aps["y"] = nc.dram_tensor("y", [B, L, HS], dt.float32, kind="ExternalOutput").ap()
    with tile.TileContext(nc, trace_sim=False) as tc:
        with ExitStack() as ctx:
            _emit_kernel(nc, tc, ctx, aps)
    nc.compile()
    _prog.nc = nc
    _prog.CF, _prog.AI = build_constants()
    _prog.built = True


def _splitH(arr_nh):
    # [N, HS] -> [(hg n), HG]: rows 0-63 = channels 0-63, rows 64-127 = 64-127
    a = np.ascontiguousarray(arr_nh, dtype=f32)
    return np.ascontiguousarray(np.concatenate([a[:, 0:HG], a[:, HG:2 * HG]], axis=0))


def make_in_maps(u, D, log_dt, W_re, W_im, Lambda_re, Lambda_im):
    _build_program()
    in_maps = []
    for c in range(NCORES):
        h0 = c * HS
        in_maps.append({
            "u": np.ascontiguousarray(u[:, :, h0:h0 + HS]).astype(bf16),
            "D": np.ascontiguousarray(D[h0:h0 + HS], dtype=f32).reshape(1, HS),
            "logdt": np.ascontiguousarray(log_dt[h0:h0 + HS], dtype=f32).reshape(1, HS),
            "Wre": _splitH(W_re[h0:h0 + HS].T),
            "Wim": _splitH(W_im[h0:h0 + HS].T),
            "Lre": np.ascontiguousarray(Lambda_re, dtype=f32).reshape(1, N),
            "Lim": np.ascontiguousarray(Lambda_im, dtype=f32).reshape(1, N),
            "CF": _prog.CF,
            "AI": _prog.AI,
        })
    return in_maps


LAST_RESULTS = None


def kernel(u, D, Lambda_re, Lambda_im, log_dt, W_re, W_im):
    global LAST_RESULTS
    from concourse.bass_utils import run_bass_kernel_spmd
    in_maps = make_in_maps(u, D, log_dt, W_re, W_im, Lambda_re, Lambda_im)
    res = run_bass_kernel_spmd(_prog.nc, in_maps, core_ids=list(range(NCORES)))
    LAST_RESULTS = res
    y = np.concatenate([res.results[c]["y"] for c in range(NCORES)], axis=2)
    return y.astype(np.float32)
